# revision 17
# baseline (speedup 1.0000x reference)
"""Trainium2 Bass kernel for a single nGPT-style attention head.

Computation (see reference): fused QKV projection, RoPE over the full head
dim, L2-normalize q/k scaled by sqk, causal SDPA with scale sqrt(d_model).

Sharding: data-parallel over batch — 8 batch elements, one per NeuronCore.
Each core gets x[b] (pre-transposed on host to [C, T] so the contraction
dim lands on SBUF partitions), the shared QKV weight (pre-transposed to
[C, 3D]), precomputed RoPE cos/sin tables, a causal triangle mask tile and
sqk. The core computes out^T = [D, T]; the host transposes back and stacks.

Device-side structure (single scheduling scope so all phases pipeline):
  per t-block j: QKV matmuls (fp32r) -> psum; copies to bf16 q^T/k^T/v^T;
  squares + partition_all_reduce (GPSIMD) for norms; ACT Sqrt; chunked RoPE
  on DVE/GPSIMD; 1/norm broadcast via GPSIMD partition_broadcast; v
  transposed via DRAM-roundtrip XBAR DMA. Then per tq-block J: scores^T
  strips (bf16 matmul) -> PSUM, ACT exp -> bf16, causal handling via
  triangle-mask multiply + zero prefix, attn@v and ones-denominator
  matmuls accumulate out^T and rowsums, final softmax division on DVE.
"""

import numpy as np
import ml_dtypes

import concourse.bass as bass
import concourse.tile as tile
from concourse import bacc, mybir, bass_isa
from concourse.bass import ts, ds
from concourse.bass_utils import run_bass_kernel_spmd

# Surface compile-hook exceptions (the PJRT bridge swallows tracebacks).
try:
    import traceback
    import libneuronxla as _lnx

    if not getattr(_lnx, "_err_wrapped", False):
        _orig_cc = _lnx.neuronx_cc

        def _cc_wrapper(*a, **kw):
            try:
                return _orig_cc(*a, **kw)
            except BaseException:
                traceback.print_exc()
                raise

        _lnx.neuronx_cc = _cc_wrapper
        _lnx._err_wrapped = True
except Exception:
    pass

AFT = mybir.ActivationFunctionType
ALU = mybir.AluOpType
F32 = mybir.dt.float32
F32R = mybir.dt.float32r
BF16 = mybir.dt.bfloat16

B, T_FULL, C, D = 8, 2048, 1024, 128
ROPE_BASE = 10000.0
P = 128
TB = 512  # t-block (tq block width, PSUM-bank free dim)
NCO = C // P  # contraction chunks for the QKV projection


def build_nc(T=T_FULL, num_devices=8):
    from contextlib import ExitStack
    NTB = T // TB
    NKT = T // P
    nc = bacc.Bacc("TRN2", target_bir_lowering=False, debug=False,
                   num_devices=num_devices)

    xT = nc.dram_tensor("xT", [C, T], F32R, kind="ExternalInput").ap()
    WT = nc.dram_tensor("WT", [C, 3 * D], F32R, kind="ExternalInput").ap()
    cosF = nc.dram_tensor("cosF", [P, 2 * T], BF16, kind="ExternalInput").ap()
    sinF = nc.dram_tensor("sinF", [P, 2 * T], BF16, kind="ExternalInput").ap()
    tri = nc.dram_tensor("tri", [P, P], BF16, kind="ExternalInput").ap()
    sqk = nc.dram_tensor("sqk", [D, 1], F32, kind="ExternalInput").ap()
    zro = nc.dram_tensor("zro", [P, 3 * P], BF16, kind="ExternalInput").ap()
    onb = nc.dram_tensor("onb", [P, 1], BF16, kind="ExternalInput").ap()
    outT = nc.dram_tensor("outT", [D, T], F32, kind="ExternalOutput").ap()

    xT_t = xT.rearrange("(co p) t -> p co t", p=P)
    WT_t = WT.rearrange("(co p) d -> p co d", p=P)
    H = P // 2

    with tile.TileContext(nc) as tc:
        with ExitStack() as ctx:
            const = ctx.enter_context(tc.tile_pool(name="const", bufs=1))
            wpool = ctx.enter_context(tc.tile_pool(name="wpool", bufs=2))
            xpool = ctx.enter_context(tc.tile_pool(name="xpool", bufs=12))
            expool = ctx.enter_context(tc.tile_pool(name="expool", bufs=3))
            dramp = ctx.enter_context(
                tc.tile_pool(name="dramp", bufs=1, space="DRAM"))
            ps_qkv = ctx.enter_context(
                tc.tile_pool(name="ps_qkv", bufs=2, space="PSUM"))
            ps_sc = ctx.enter_context(
                tc.tile_pool(name="ps_sc", bufs=2, space="PSUM"))
            ps_o = ctx.enter_context(
                tc.tile_pool(name="ps_o", bufs=2, space="PSUM"))
            ps_d = ctx.enter_context(
                tc.tile_pool(name="ps_d", bufs=2, space="PSUM"))

            wt = const.tile([P, NCO, 3 * D], F32R)
            nc.sync.dma_start(wt, WT_t)
            sqk_sb = const.tile([D, 1], F32)
            nc.sync.dma_start(sqk_sb, sqk)
            ones_k = const.tile([P, 1], BF16)
            nc.sync.dma_start(ones_k, onb)
            tri_sb = const.tile([P, P], BF16)
            nc.sync.dma_start(tri_sb, tri)
            # (sqk * C^(1/4))^2 = sqrt(C) * sqk^2 — full logit scale, on q.
            sqk232 = const.tile([D, 1], F32)
            nc.vector.tensor_scalar_mul(sqk232, sqk_sb, float(C ** 0.25))
            nc.vector.tensor_mul(sqk232, sqk232, sqk232)

            cos_sb = const.tile([P, 2 * T], BF16)
            sin_sb = const.tile([P, 2 * T], BF16)
            qk = const.tile([P, 2 * T], BF16)   # q̃^T | k̃^T
            vst = const.tile([P, T], BF16)      # v^T staging
            vt = const.tile([P, NKT, P], BF16)  # v tiles [tk, e]
            nrm = const.tile([1, 2 * T], F32)
            invn = const.tile([1, 2 * T], F32)
            vd = dramp.tile([P, T], BF16)

            # ---------- per-block: QKV + norms + RoPE + v ----------
            for j in range(NTB):
                with nc.named_scope(f"qkv{j}"):
                    xts = []
                    for co in range(NCO):
                        xt = xpool.tile([P, TB], F32R, tag="xt")
                        nc.sync.dma_start(xt, xT_t[:, co, ts(j, TB)])
                        xts.append(xt)
                    for g in range(3):
                        ps = ps_qkv.tile([P, TB], F32, tag="qkv")
                        for co in range(NCO):
                            nc.tensor.matmul(
                                ps, wt[:, co, ts(g, D)], xts[co],
                                start=(co == 0), stop=(co == NCO - 1))
                        if g < 2:
                            dst = qk[:, ds(g * T + j * TB, TB)]
                            nc.any.tensor_copy(out=dst, in_=ps)
                            sq = wpool.tile([P, TB], F32, tag="sq")
                            nc.gpsimd.tensor_mul(sq, dst, dst)
                            par = wpool.tile([P, TB], F32, tag="par")
                            nc.gpsimd.partition_all_reduce(
                                par, sq, P, bass_isa.ReduceOp.add)
                            nc.scalar.activation(
                                nrm[:, ds(g * T + j * TB, TB)],
                                par[0:1, :], AFT.Sqrt)
                        else:
                            nc.any.tensor_copy(
                                out=vst[:, ds(j * TB, TB)], in_=ps)

                with nc.named_scope(f"rope{j}"):
                    for part in range(2):  # 0 = q chunk, 1 = k chunk
                        ofs = part * T + j * TB
                        ch = ds(ofs, TB)
                        nc.sync.dma_start(cos_sb[:, ch], cosF[:, ch])
                        nc.sync.dma_start(sin_sb[:, ch], sinF[:, ch])
                        rot = wpool.tile([P, TB], BF16, tag="rot")
                        nc.vector.tensor_scalar_mul(
                            rot[0:H, :], qk[H:P, ch], -1.0)
                        nc.vector.tensor_copy(rot[H:P, :], qk[0:H, ch])
                        t1 = wpool.tile([P, TB], BF16, tag="t1")
                        nc.vector.tensor_mul(t1, qk[:, ch], cos_sb[:, ch])
                        t2 = wpool.tile([P, TB], BF16, tag="t2")
                        nc.gpsimd.tensor_mul(t2, rot, sin_sb[:, ch])
                        nc.vector.tensor_add(t1, t1, t2)
                        chn = ds(ofs, TB)
                        nc.vector.reciprocal(invn[:, chn], nrm[:, chn])
                        bcst = wpool.tile([P, TB], F32, tag="bcst")
                        nc.gpsimd.partition_broadcast(bcst, invn[:, chn])
                        if part == 0:
                            nc.vector.scalar_tensor_tensor(
                                out=qk[:, ch], in0=t1, scalar=sqk232,
                                in1=bcst, op0=ALU.mult, op1=ALU.mult)
                        else:
                            nc.vector.tensor_mul(qk[:, ch], t1, bcst)

                    # v transpose via DRAM-roundtrip XBAR DMA (bf16)
                    nc.sync.dma_start(vd[:, ts(j, TB)], vst[:, ts(j, TB)])
                    for i in range(4 * j, 4 * j + 4):
                        nc.sync.dma_start_transpose(vt[:, i, :],
                                                    vd[:, ts(i, P)])

            # ---------- attention per tq-block ----------
            for J in range(NTB):
                with nc.named_scope(f"att{J}"):
                    q_blk = qk[:, ts(J, TB)]
                    po = ps_o.tile([P, TB], F32, tag="o")
                    pd = ps_d.tile([1, TB], F32, tag="d")
                    nstr = (TB // P) * (J + 1)
                    for i in range(nstr):
                        dr = i - (TB // P) * J  # >=0 on diagonal strips
                        off = P * dr if dr >= 0 else 0
                        sc = ps_sc.tile([P, TB], F32, tag="sc")
                        ex = expool.tile([P, TB], BF16, tag="ex")
                        nc.tensor.matmul(
                            sc[:, ds(off, TB - off)],
                            qk[:, ds(T + P * i, P)],
                            q_blk[:, ds(off, TB - off)],
                            start=True, stop=True)
                        nc.scalar.activation(
                            ex[:, ds(off, TB - off)],
                            sc[:, ds(off, TB - off)], AFT.Exp)
                        if dr >= 0:
                            nc.gpsimd.tensor_mul(
                                ex[:, ds(off, P)], ex[:, ds(off, P)], tri_sb)
                            if off > 0:
                                nc.sync.dma_start(ex[:, ds(0, off)],
                                                  zro[:, ds(0, off)])
                        nc.tensor.matmul(po, vt[:, i, :], ex,
                                         start=(i == 0), stop=(i == nstr - 1))
                        nc.tensor.matmul(pd, ones_k, ex,
                                         start=(i == 0), stop=(i == nstr - 1))

                    invd = wpool.tile([1, TB], F32, tag="invd")
                    nc.vector.reciprocal(invd, pd)
                    bc2s = wpool.tile([P, TB], F32, tag="bc2s")
                    nc.gpsimd.partition_broadcast(bc2s, invd)
                    ob = wpool.tile([P, TB], F32, tag="ob")
                    nc.vector.tensor_mul(ob, po, bc2s)
                    nc.sync.dma_start(outT[:, ts(J, TB)], ob)

    nc.compile()
    return nc


def _host_tables(T):
    d = D
    inv_freq = 1.0 / (ROPE_BASE ** (np.arange(0, d, 2, dtype=np.float64) / d))
    t = np.arange(T, dtype=np.float64)
    freqs = np.outer(inv_freq, t)  # [d/2, T]
    emb = np.concatenate([freqs, freqs], axis=0)  # [d, T]
    cos1 = np.cos(emb)
    sin1 = np.sin(emb)
    cosF = np.concatenate([cos1, cos1], axis=1).astype(ml_dtypes.bfloat16)
    sinF = np.concatenate([sin1, sin1], axis=1).astype(ml_dtypes.bfloat16)
    a = np.arange(P)
    tri = (a[None, :] >= a[:, None]).astype(ml_dtypes.bfloat16)  # [tk, tq]
    return cosF, sinF, tri


TRACE = False
LAST_EXEC_NS = None
LAST_TRACE = None
LAST_INSTS = None


def kernel(x, W_qkv, sqk):
    global LAST_EXEC_NS, LAST_TRACE, LAST_INSTS
    T = x.shape[1]
    cosF, sinF, tri = _host_tables(T)
    WT = np.ascontiguousarray(np.asarray(W_qkv).T).astype(np.float32)
    sqk2 = np.ascontiguousarray(np.asarray(sqk).reshape(D, 1)).astype(np.float32)
    in_maps = []
    for b in range(B):
        in_maps.append({
            "xT": np.ascontiguousarray(np.asarray(x[b]).T).astype(np.float32),
            "WT": WT,
            "cosF": cosF,
            "sinF": sinF,
            "tri": tri,
            "sqk": sqk2,
            "zro": np.zeros((P, 3 * P), ml_dtypes.bfloat16),
            "onb": np.ones((P, 1), ml_dtypes.bfloat16),
        })
    nc = build_nc(T=T, num_devices=B)
    res = run_bass_kernel_spmd(nc, in_maps, core_ids=list(range(B)),
                               trace=TRACE)
    LAST_EXEC_NS = res.exec_time_ns
    LAST_TRACE = (res.instructions_and_trace[1]
                  if res.instructions_and_trace else None)
    LAST_INSTS = (res.instructions_and_trace[0]
                  if res.instructions_and_trace else None)
    out = np.stack([r["outT"].T for r in res.results])  # [B, T, D]
    return np.ascontiguousarray(out).astype(np.float32)
